# revision 3
# baseline (speedup 1.0000x reference)
"""GCN layer kernel for 8 trn2 NeuronCores.

Math:  out = D (A + I) D feature W^T + b      (D = diag(hat_d))
Rewritten with g = (hat_d * feature) @ W^T  (the linear commutes with the
row-scaling and the SpMM):
    out = hat_d * (A @ g) + hat_d * g + b

Sharding: A row-sharded across 8 cores (2048 rows each). Each core
computes full g locally from a replicated feature^T (N*d is small vs N^2,
so replicating this small compute is cheaper than a collective), then
streams its A-shard once for the big matmul.

Device layout: the big matmul is computed transposed,
out_sh^T[o, m] = sum_j g[j, o] * A_sh^T[j, m], so g tiles are the
stationary operand and the A shard (pre-transposed on the host — lhsT
layout prep for the systolic array) is the moving operand in natural
layout. The host applies an "own rows first" node permutation to the j
axis of A^T / feature^T / hat_d so the same SPMD program works on every
core (own-shard g tiles are always j = 0..15).

A is stored fp8 e3m4 of (A - 0.5): the mean-subtraction centers the
uniform[0,1) entries so the relative fp8 error applies to half the
magnitude, and the exact rank-1 term 0.5 * colsum(g) is added back in
the epilogue (colsum accumulated on DVE during phase 1 + two 1-column
matmuls). This halves A's HBM traffic vs fp16; measured end-to-end
relative error ~8e-3. g / feature / W stay fp16 (their quantization
error propagates ~1:1 to the output, fp8 there would fail the 2e-2
budget). The PE runs the mixed fp16(stationary) x fp8(moving) matmul at
the full 16-bit rate through the fp22 datapath.

The main matmul is split into two m-chunks of 1024 so the first chunk's
epilogue (DVE scale/add + DMA out) overlaps the second chunk's matmul
stream instead of serializing after it.

Matmul accumulation and the epilogue stay fp32.
"""

import os

import ml_dtypes
import numpy as np

import concourse.mybir as mybir
import concourse.tile as tile
from concourse import bacc
from concourse.bass_utils import run_bass_kernel_spmd
from concourse.masks import make_identity

N = 16384
F = 512  # in features
O = 256  # out features
NCORES = 8
SH = N // NCORES  # 2048 rows per core
JT = N // 128  # 128 node tiles
MT = SH // 128  # 16 own node tiles
NB = 2048  # phase-1 node-block width (per feature slab)
MC = 1024  # main-matmul m-chunk width
NCH = SH // MC  # 2 m-chunks

F32 = mybir.dt.float32
F16 = mybir.dt.float16
F8 = mybir.dt.float8e3  # e3m4

_CACHE = {}


def build_program():
    nc = bacc.Bacc("TRN2", target_bir_lowering=False, debug=False,
                   num_devices=NCORES, dynamic_dma_scratch_size=8192)

    at = nc.dram_tensor("at", [N, SH], F8, kind="ExternalInput").ap()
    ft = nc.dram_tensor("ft", [F, N], F16, kind="ExternalInput").ap()
    hdt = nc.dram_tensor("hdt", [128, JT], F32, kind="ExternalInput").ap()
    hdo = nc.dram_tensor("hdo", [1, SH], F32, kind="ExternalInput").ap()
    wt = nc.dram_tensor("wt", [F, O], F16, kind="ExternalInput").ap()
    bvec = nc.dram_tensor("bvec", [O, 1], F32, kind="ExternalInput").ap()
    outT = nc.dram_tensor("outT", [O, SH], F32, kind="ExternalOutput").ap()

    add = mybir.AluOpType.add
    mult = mybir.AluOpType.mult

    with tile.TileContext(nc) as tc:
        with (
            tc.tile_pool(name="const", bufs=1) as constp,
            tc.tile_pool(name="gpool", bufs=1) as gp,
            tc.tile_pool(name="fslab", bufs=12) as fsp,
            tc.tile_pool(name="aslab", bufs=8) as asp,
            tc.tile_pool(name="tout", bufs=4) as wp,
            tc.tile_pool(name="scr", bufs=2) as scp,
        ):
            qs = [nc.sync, nc.scalar]

            # First feature block is loaded as two half-width slab groups
            # (the first matmul then waits on a 256KB transfer, not 512KB)
            # and goes out before the consts so phase-1 starts early.
            half_slabs = [[], []]
            for hb in range(2):
                for fc in range(4):
                    s = fsp.tile([128, NB // 2], F16, tag="fs",
                                 name=f"fs0{hb}_{fc}")
                    qs[fc % 2].dma_start(
                        out=s[:],
                        in_=ft[fc * 128:(fc + 1) * 128,
                               hb * (NB // 2):(hb + 1) * (NB // 2)])
                    half_slabs[hb].append(s)

            ident = constp.tile([128, 128], F32, tag="ident")
            make_identity(nc, ident[:])

            wt_sb = constp.tile([128, 4 * O], F16, tag="wt")
            for fc in range(4):
                nc.scalar.dma_start(out=wt_sb[:, fc * O:(fc + 1) * O],
                                    in_=wt[fc * 128:(fc + 1) * 128, :])
            hdt_sb = constp.tile([128, JT], F32, tag="hdt")
            nc.scalar.dma_start(out=hdt_sb[:], in_=hdt[:, :])
            halfones = constp.tile([128, 1], F16, tag="halfones")
            nc.gpsimd.memset(halfones[:], 0.5)

            # g for all nodes (fp16), node-tile j at columns [j*O, (j+1)*O)
            g_sb = gp.tile([128, JT * O], F16, tag="g")
            # e = (hat_d_own * g_own)^T (fp32), o-half h at cols [h*SH, (h+1)*SH)
            e_sb = gp.tile([128, 2 * SH], F32, tag="e")
            # running colsum of g over node tiles (partition = j-in-tile)
            gsum = gp.tile([128, O], F32, tag="gsum")
            gsum16 = gp.tile([128, O], F16, tag="gsum16")
            # corr[o] = 0.5 * sum_j g[j, o], o-half h in column h
            corr_sb = constp.tile([128, 2], F32, tag="corr")

            # ---- phase 1: g = (hat_d * feature) @ W^T for all nodes ----
            # Own PSUM pool (closed before the accumulators are allocated)
            # so the fw tiles pipeline 6 deep instead of sharing the
            # accumulator-sized slots. The hat_d row-scale alternates
            # between DVE and ACT so neither engine gates the PE stream.
            with tc.tile_pool(name="ps1", bufs=2, space="PSUM") as ps1:
                for jb in range(N // NB):
                    if jb == 0:
                        slabs = None  # handled per-jj via half_slabs
                    else:
                        slabs = []
                        for fc in range(4):
                            s = fsp.tile([128, NB], F16, tag="fs",
                                         name=f"fs{jb}_{fc}")
                            qs[fc % 2].dma_start(
                                out=s[:],
                                in_=ft[fc * 128:(fc + 1) * 128,
                                       jb * NB:(jb + 1) * NB])
                            slabs.append(s)
                    for jj in range(NB // 128):
                        j = jb * (NB // 128) + jj
                        if jb == 0:
                            sl_group = half_slabs[jj // 8]
                            col = (jj % 8) * 128
                        else:
                            sl_group = slabs
                            col = jj * 128
                        pfw = ps1.tile([128, O], F32, tag="fw", bufs=6)
                        for fc in range(4):
                            nc.tensor.matmul(
                                pfw[:],
                                lhsT=sl_group[fc][:, col:col + 128],
                                rhs=wt_sb[:, fc * O:(fc + 1) * O],
                                start=(fc == 0), stop=(fc == 3))
                        gt = g_sb[:, j * O:(j + 1) * O]
                        if j % 2 == 0:
                            nc.vector.tensor_scalar_mul(
                                gt, pfw[:], hdt_sb[:, j:j + 1])
                        else:
                            nc.scalar.mul(
                                gt, pfw[:], hdt_sb[:, j:j + 1])
                        # colsum accumulation on DVE (idle vs the PE stream)
                        if j == 0:
                            nc.vector.tensor_copy(gsum[:], gt)
                        else:
                            nc.vector.tensor_tensor(gsum[:], gsum[:], gt, add)

                    if jb == 0:
                        # e = (hat_d_own * g_own)^T; own tiles are j =
                        # 0..MT-1, all inside block 0. Runs while later
                        # blocks stream in.
                        for jj in range(MT):
                            for h in range(2):
                                sc = scp.tile([128, 128], F32, tag="sc")
                                nc.vector.tensor_scalar_mul(
                                    sc[:],
                                    g_sb[:, jj * O + h * 128:
                                         jj * O + (h + 1) * 128],
                                    hdt_sb[:, jj:jj + 1])
                                ptp = ps1.tile([128, 128], F32, tag="tp",
                                               bufs=2)
                                nc.tensor.transpose(ptp[:], sc[:], ident[:])
                                nc.vector.tensor_copy(
                                    e_sb[:, h * SH + jj * 128:
                                         h * SH + (jj + 1) * 128],
                                    ptp[:])

                # corr = 0.5 * colsum(g): partition-reduce gsum via two
                # 1-column matmuls against a 0.5-filled vector.
                nc.vector.tensor_copy(gsum16[:], gsum[:])
                for h in range(2):
                    pcorr = ps1.tile([128, 128], F32, tag="tp", bufs=2)
                    nc.tensor.matmul(
                        pcorr[:, 0:1],
                        lhsT=gsum16[:, h * 128:(h + 1) * 128],
                        rhs=halfones[:],
                        start=True, stop=True)
                    nc.vector.tensor_copy(corr_sb[:, h:h + 1], pcorr[:, 0:1])

            # epilogue-only constants: queued between the feature stream and
            # the A stream; done long before the epilogue needs them.
            b_sb = constp.tile([128, 2], F32, tag="b")
            for h in range(2):
                nc.scalar.dma_start(out=b_sb[:, h:h + 1],
                                    in_=bvec[h * 128:(h + 1) * 128, :])
            # hat_d of own rows broadcast across all partitions (free dim = m)
            hd_bc = constp.tile([128, SH], F32, tag="hdbc")
            nc.scalar.dma_start(out=hd_bc[:],
                                in_=hdo[0:1, :].to_broadcast((128, SH)))

            # ---- main: acc[h] = (A_sh @ g)^T for o-half h, two m-chunks ----
            # Chunking lets chunk 0's epilogue overlap chunk 1's matmuls.
            with tc.tile_pool(name="ps2", bufs=2, space="PSUM") as psp:
                for c in range(NCH):
                    accs = [psp.tile([128, MC], F32, tag=f"acc{h}",
                                     name=f"acc{c}_{h}", bufs=2)
                            for h in range(2)]
                    for k in range(JT):
                        sl = asp.tile([128, MC], F8, tag="as")
                        qs[k % 2].dma_start(
                            out=sl[:],
                            in_=at[k * 128:(k + 1) * 128,
                                   c * MC:(c + 1) * MC])
                        for h in range(2):
                            lhs = g_sb[:, k * O + h * 128:
                                       k * O + (h + 1) * 128]
                            for mc in range(MC // 512):
                                nc.tensor.matmul(
                                    accs[h][:, mc * 512:(mc + 1) * 512],
                                    lhsT=lhs,
                                    rhs=sl[:, mc * 512:(mc + 1) * 512],
                                    start=(k == 0), stop=(k == JT - 1))

                    # epilogue: out^T = hat_d_own * (acc + corr) + b + e
                    for h in range(2):
                        for sub in range(MC // 512):
                            ms = c * MC + sub * 512
                            t = wp.tile([128, 512], F32, tag="t")
                            nc.vector.scalar_tensor_tensor(
                                t[:], in0=accs[h][:, sub * 512:(sub + 1) * 512],
                                scalar=corr_sb[:, h:h + 1],
                                in1=hd_bc[:, ms:ms + 512],
                                op0=add, op1=mult)
                            nc.vector.scalar_tensor_tensor(
                                t[:], in0=t[:], scalar=b_sb[:, h:h + 1],
                                in1=e_sb[:, h * SH + ms:h * SH + ms + 512],
                                op0=add, op1=add)
                            qs[(h + sub) % 2].dma_start(
                                out=outT[h * 128:(h + 1) * 128, ms:ms + 512],
                                in_=t[:])

    nc.compile()
    return nc


def prep_inputs(A, hat_d, feature, W, b):
    """Per-core input maps. Host work is layout/dtype prep only: transpose,
    slice, concatenate (the own-rows-first node permutation on the j axis),
    and the fp32->fp8/fp16 dtype conversion for matmul operands (A is
    quantized as fp8(A - 0.5); the 0.5 offset is restored on-device)."""
    A = np.ascontiguousarray(np.asarray(A, dtype=np.float32))
    hat_d = np.ascontiguousarray(np.asarray(hat_d, dtype=np.float32))
    feature = np.ascontiguousarray(np.asarray(feature, dtype=np.float32))
    W = np.asarray(W, dtype=np.float32)
    b = np.asarray(b, dtype=np.float32)

    featT = np.ascontiguousarray(feature.T.astype(np.float16))  # [F, N]
    wt = np.ascontiguousarray(W.T.astype(np.float16))  # [F, O]
    b2 = np.ascontiguousarray(b.reshape(O, 1))

    FP8 = ml_dtypes.float8_e3m4
    in_maps = []
    for c in range(NCORES):
        r0, r1 = c * SH, (c + 1) * SH
        rows = (A[r0:r1] - np.float32(0.5)).astype(FP8)  # [SH, N] fp8
        # A_sh^T with node (j) axis permuted own-rows-first
        at_c = np.empty((N, SH), dtype=FP8)
        at_c[:SH] = rows[:, r0:r1].T
        at_c[SH:SH + r0] = rows[:, :r0].T
        at_c[SH + r0:] = rows[:, r1:].T

        ft_c = np.empty((F, N), dtype=np.float16)
        ft_c[:, :SH] = featT[:, r0:r1]
        ft_c[:, SH:SH + r0] = featT[:, :r0]
        ft_c[:, SH + r0:] = featT[:, r1:]

        hd_c = np.concatenate([hat_d[r0:r1], hat_d[:r0], hat_d[r1:]])
        hdt_c = np.ascontiguousarray(hd_c.reshape(JT, 128).T)
        hdo_c = np.ascontiguousarray(hat_d[r0:r1].reshape(1, SH))

        in_maps.append({
            "at": at_c,
            "ft": ft_c,
            "hdt": hdt_c,
            "hdo": hdo_c,
            "wt": wt,
            "bvec": b2,
        })
    return in_maps


last_exec_time_ns = None
last_results = None


def kernel(A, hat_d, feature, W, b):
    global last_exec_time_ns, last_results
    if "nc" not in _CACHE:
        _CACHE["nc"] = build_program()
    nc = _CACHE["nc"]

    in_maps = prep_inputs(A, hat_d, feature, W, b)
    trace = bool(int(os.environ.get("KERNEL_TRACE", "0")))
    res = run_bass_kernel_spmd(nc, in_maps, list(range(NCORES)), trace=trace)
    last_exec_time_ns = res.exec_time_ns
    last_results = res

    out = np.empty((N, O), dtype=np.float32)
    for c in range(NCORES):
        out[c * SH:(c + 1) * SH] = res.results[c]["outT"].T
    return out


# revision 5
# speedup vs baseline: 1.0454x; 1.0454x over previous
"""GCN layer kernel for 8 trn2 NeuronCores.

Math:  out = D (A + I) D feature W^T + b      (D = diag(hat_d))
Rewritten with g = (hat_d * feature) @ W^T  (the linear commutes with the
row-scaling and the SpMM):
    out = hat_d * (A @ g) + hat_d * g + b

Sharding: A row-sharded across 8 cores (2048 rows each). Each core
computes full g locally from a replicated feature^T (N*d is small vs N^2,
so replicating this small compute is cheaper than a collective), then
streams its A-shard once for the big matmul.

Device layout: the big matmul is computed transposed,
out_sh^T[o, m] = sum_j g[j, o] * A_sh^T[j, m], so g tiles are the
stationary operand and the A shard (pre-transposed on the host — lhsT
layout prep for the systolic array) is the moving operand in natural
layout. The host applies an "own rows first" node permutation to the j
axis of A^T / feature^T / hat_d so the same SPMD program works on every
core (own-shard g tiles are always j = 0..15).

A is stored fp8 e3m4 of (A - 0.5): the mean-subtraction centers the
uniform[0,1) entries so the fp8 relative error applies to half the
magnitude, and the exact rank-1 term 0.5 * colsum(g) is added back in
the epilogue. colsum(g) is accumulated on DVE with one [128, 4096] fold
per feature block (idle-engine work, ~2us each) plus a 4-op tree and two
1-column matmuls at the end, so it adds ~4us to the critical path. This
halves A's HBM traffic vs fp16; measured end-to-end relative error
~8e-3. g / feature / W stay fp16 (their error propagates ~1:1 to the
output; fp8 there would blow the 2e-2 budget). The PE runs the mixed
fp16(stationary) x fp8(moving) matmul at the full 16-bit rate.

The main matmul is split into two m-chunks of 1024 so the first chunk's
epilogue (DVE scale/add + DMA out) overlaps the second chunk's matmul
stream. A is host-packed into [chunk][k-oct] blocks so each chunk's
stream is 32 x 1MB contiguous DMAs (8 node tiles per transfer) instead
of 256 small ones. The output is written fp16 (host upcasts) to halve
the tail DMA.
"""

import os

import ml_dtypes
import numpy as np

import concourse.mybir as mybir
import concourse.tile as tile
from concourse import bacc
from concourse.bass_utils import run_bass_kernel_spmd
from concourse.masks import make_identity

N = 16384
F = 512  # in features
O = 256  # out features
NCORES = 8
SH = N // NCORES  # 2048 rows per core
JT = N // 128  # 128 node tiles
MT = SH // 128  # 16 own node tiles
NB = 2048  # phase-1 node-block width (per feature slab)
MC = 1024  # main-matmul m-chunk width
NCH = SH // MC  # 2 m-chunks
KB = 8  # node tiles per A DMA block (1MB transfers)

F32 = mybir.dt.float32
F16 = mybir.dt.float16
F8 = mybir.dt.float8e3  # e3m4

_CACHE = {}


def build_program():
    nc = bacc.Bacc("TRN2", target_bir_lowering=False, debug=False,
                   num_devices=NCORES, dynamic_dma_scratch_size=8192)

    at = nc.dram_tensor("at", [NCH, JT // KB, 128, KB * MC], F8,
                        kind="ExternalInput").ap()
    ft = nc.dram_tensor("ft", [F, N], F16, kind="ExternalInput").ap()
    hdt = nc.dram_tensor("hdt", [128, JT], F32, kind="ExternalInput").ap()
    hdo = nc.dram_tensor("hdo", [1, SH], F32, kind="ExternalInput").ap()
    wt = nc.dram_tensor("wt", [F, O], F16, kind="ExternalInput").ap()
    bvec = nc.dram_tensor("bvec", [O, 1], F32, kind="ExternalInput").ap()
    outT = nc.dram_tensor("outT", [O, SH], F16, kind="ExternalOutput").ap()

    add = mybir.AluOpType.add
    mult = mybir.AluOpType.mult

    with tile.TileContext(nc) as tc:
        with (
            tc.tile_pool(name="const", bufs=1) as constp,
            tc.tile_pool(name="gpool", bufs=1) as gp,
            tc.tile_pool(name="fslab", bufs=12) as fsp,
            tc.tile_pool(name="aslab", bufs=3) as asp,
            tc.tile_pool(name="tout", bufs=4) as wp,
            tc.tile_pool(name="scr", bufs=2) as scp,
        ):
            qs = [nc.sync, nc.scalar]
            qs4 = [nc.sync, nc.scalar, nc.gpsimd, nc.sync]

            # First feature block is loaded as two half-width slab groups
            # across four queues (the first matmul then waits on a 256KB
            # transfer) and goes out before the consts so phase-1 starts
            # early.
            half_slabs = [[], []]
            for hb in range(2):
                for fc in range(4):
                    s = fsp.tile([128, NB // 2], F16, tag="fs",
                                 name=f"fs0{hb}_{fc}")
                    qs4[fc].dma_start(
                        out=s[:],
                        in_=ft[fc * 128:(fc + 1) * 128,
                               hb * (NB // 2):(hb + 1) * (NB // 2)])
                    half_slabs[hb].append(s)

            ident = constp.tile([128, 128], F32, tag="ident")
            make_identity(nc, ident[:])

            wt_sb = constp.tile([128, 4 * O], F16, tag="wt")
            for fc in range(4):
                nc.scalar.dma_start(out=wt_sb[:, fc * O:(fc + 1) * O],
                                    in_=wt[fc * 128:(fc + 1) * 128, :])
            hdt_sb = constp.tile([128, JT], F32, tag="hdt")
            nc.scalar.dma_start(out=hdt_sb[:], in_=hdt[:, :])
            halfones = constp.tile([128, 1], F32, tag="halfones")
            nc.gpsimd.memset(halfones[:], 0.5)

            # g for all nodes (fp16), node-tile j at columns [j*O, (j+1)*O)
            g_sb = gp.tile([128, JT * O], F16, tag="g")
            # e = (hat_d_own * g_own)^T (fp32), o-half h at cols [h*SH, (h+1)*SH)
            e_sb = gp.tile([128, 2 * SH], F32, tag="e")
            # per-block colsum accumulator (fp32), folded 4096 -> 256 at end
            gsum = gp.tile([128, 16 * O], F32, tag="gsum")
            # corr[o] = 0.5 * sum_j g[j, o], o-half h in column h
            corr_sb = constp.tile([128, 2], F32, tag="corr")

            # ---- phase 1: g = (hat_d * feature) @ W^T for all nodes ----
            # Own PSUM pool (closed before the accumulators are allocated)
            # so the fw tiles pipeline 6 deep instead of sharing the
            # accumulator-sized slots. The hat_d row-scale alternates
            # between DVE and ACT so neither engine gates the PE stream.
            with tc.tile_pool(name="ps1", bufs=2, space="PSUM") as ps1:
                for jb in range(N // NB):
                    if jb == 0:
                        slabs = None  # handled per-jj via half_slabs
                    else:
                        slabs = []
                        for fc in range(4):
                            s = fsp.tile([128, NB], F16, tag="fs",
                                         name=f"fs{jb}_{fc}")
                            qs[fc % 2].dma_start(
                                out=s[:],
                                in_=ft[fc * 128:(fc + 1) * 128,
                                       jb * NB:(jb + 1) * NB])
                            slabs.append(s)
                    for jj in range(NB // 128):
                        j = jb * (NB // 128) + jj
                        if jb == 0:
                            sl_group = half_slabs[jj // 8]
                            col = (jj % 8) * 128
                        else:
                            sl_group = slabs
                            col = jj * 128
                        pfw = ps1.tile([128, O], F32, tag="fw", bufs=6)
                        for fc in range(4):
                            nc.tensor.matmul(
                                pfw[:],
                                lhsT=sl_group[fc][:, col:col + 128],
                                rhs=wt_sb[:, fc * O:(fc + 1) * O],
                                start=(fc == 0), stop=(fc == 3))
                        gt = g_sb[:, j * O:(j + 1) * O]
                        if j % 2 == 0:
                            nc.vector.tensor_scalar_mul(
                                gt, pfw[:], hdt_sb[:, j:j + 1])
                        else:
                            nc.scalar.mul(
                                gt, pfw[:], hdt_sb[:, j:j + 1])

                    # fold this block's 16 g tiles into the colsum
                    # accumulator: one [128, 4096] DVE op per block.
                    blk = g_sb[:, jb * 16 * O:(jb + 1) * 16 * O]
                    if jb == 0:
                        nc.vector.tensor_copy(gsum[:], blk)
                    else:
                        nc.vector.tensor_tensor(gsum[:], gsum[:], blk, add)

                    if jb == 0:
                        # e = (hat_d_own * g_own)^T; own tiles are j =
                        # 0..MT-1, all inside block 0. Runs while later
                        # blocks stream in.
                        for jj in range(MT):
                            for h in range(2):
                                sc = scp.tile([128, 128], F32, tag="sc")
                                nc.vector.tensor_scalar_mul(
                                    sc[:],
                                    g_sb[:, jj * O + h * 128:
                                         jj * O + (h + 1) * 128],
                                    hdt_sb[:, jj:jj + 1])
                                ptp = ps1.tile([128, 128], F32, tag="tp",
                                               bufs=2)
                                nc.tensor.transpose(ptp[:], sc[:], ident[:])
                                nc.vector.tensor_copy(
                                    e_sb[:, h * SH + jj * 128:
                                         h * SH + (jj + 1) * 128],
                                    ptp[:])

                # tree-fold gsum 4096 -> 256, then corr = 0.5 * colsum via
                # two 1-column fp32 matmuls (partition reduction on the PE).
                w = 8 * O
                while w >= O:
                    nc.vector.tensor_tensor(
                        gsum[:, :w], gsum[:, :w], gsum[:, w:2 * w], add)
                    w //= 2
                for h in range(2):
                    pcorr = ps1.tile([128, 128], F32, tag="tp", bufs=2)
                    nc.tensor.matmul(
                        pcorr[:, 0:1],
                        lhsT=gsum[:, h * 128:(h + 1) * 128],
                        rhs=halfones[:],
                        start=True, stop=True)
                    nc.vector.tensor_copy(corr_sb[:, h:h + 1], pcorr[:, 0:1])

            # epilogue-only constants: queued between the feature stream and
            # the A stream; done long before the epilogue needs them.
            b_sb = constp.tile([128, 2], F32, tag="b")
            for h in range(2):
                nc.scalar.dma_start(out=b_sb[:, h:h + 1],
                                    in_=bvec[h * 128:(h + 1) * 128, :])
            # hat_d of own rows broadcast across all partitions (free dim = m)
            hd_bc = constp.tile([128, SH], F32, tag="hdbc")
            nc.scalar.dma_start(out=hd_bc[:],
                                in_=hdo[0:1, :].to_broadcast((128, SH)))

            # ---- main: acc[h] = (A_sh @ g)^T for o-half h, two m-chunks ----
            # Chunking lets chunk 0's epilogue overlap chunk 1's matmuls.
            with tc.tile_pool(name="ps2", bufs=2, space="PSUM") as psp:
                for c in range(NCH):
                    accs = [psp.tile([128, MC], F32, tag=f"acc{h}",
                                     name=f"acc{c}_{h}", bufs=2)
                            for h in range(2)]
                    for k in range(JT):
                        if k % KB == 0:
                            sl8 = asp.tile([128, KB * MC], F8, tag="as")
                            qs[(k // KB) % 2].dma_start(
                                out=sl8[:], in_=at[c, k // KB, :, :])
                        q0 = (k % KB) * MC
                        for h in range(2):
                            lhs = g_sb[:, k * O + h * 128:
                                       k * O + (h + 1) * 128]
                            for mc in range(MC // 512):
                                nc.tensor.matmul(
                                    accs[h][:, mc * 512:(mc + 1) * 512],
                                    lhsT=lhs,
                                    rhs=sl8[:, q0 + mc * 512:
                                            q0 + (mc + 1) * 512],
                                    start=(k == 0), stop=(k == JT - 1))

                    # epilogue: out^T = hat_d_own * (acc + corr) + b + e
                    for h in range(2):
                        for sub in range(MC // 512):
                            ms = c * MC + sub * 512
                            t = wp.tile([128, 512], F32, tag="t")
                            nc.vector.scalar_tensor_tensor(
                                t[:], in0=accs[h][:, sub * 512:(sub + 1) * 512],
                                scalar=corr_sb[:, h:h + 1],
                                in1=hd_bc[:, ms:ms + 512],
                                op0=add, op1=mult)
                            t16 = wp.tile([128, 512], F16, tag="t16")
                            nc.vector.scalar_tensor_tensor(
                                t16[:], in0=t[:], scalar=b_sb[:, h:h + 1],
                                in1=e_sb[:, h * SH + ms:h * SH + ms + 512],
                                op0=add, op1=add)
                            qs[(h + sub) % 2].dma_start(
                                out=outT[h * 128:(h + 1) * 128, ms:ms + 512],
                                in_=t16[:])

    nc.compile()
    return nc


def prep_inputs(A, hat_d, feature, W, b):
    """Per-core input maps. Host work is layout/dtype prep only: transpose,
    slice, concatenate (the own-rows-first node permutation on the j axis,
    plus the chunk/k-oct DMA blocking of A), and the fp32->fp8/fp16 dtype
    conversion for matmul operands (A is quantized as fp8(A - 0.5); the
    0.5 offset is restored on-device)."""
    A = np.ascontiguousarray(np.asarray(A, dtype=np.float32))
    hat_d = np.ascontiguousarray(np.asarray(hat_d, dtype=np.float32))
    feature = np.ascontiguousarray(np.asarray(feature, dtype=np.float32))
    W = np.asarray(W, dtype=np.float32)
    b = np.asarray(b, dtype=np.float32)

    featT = np.ascontiguousarray(feature.T.astype(np.float16))  # [F, N]
    wt = np.ascontiguousarray(W.T.astype(np.float16))  # [F, O]
    b2 = np.ascontiguousarray(b.reshape(O, 1))

    FP8 = ml_dtypes.float8_e3m4
    in_maps = []
    for c in range(NCORES):
        r0, r1 = c * SH, (c + 1) * SH
        rows = (A[r0:r1] - np.float32(0.5)).astype(FP8)  # [SH, N] fp8
        # A_sh^T with node (j) axis permuted own-rows-first
        at_c = np.empty((N, SH), dtype=FP8)
        at_c[:SH] = rows[:, r0:r1].T
        at_c[SH:SH + r0] = rows[:, :r0].T
        at_c[SH + r0:] = rows[:, r1:].T
        # blocked for the DMA stream: [chunk][k-oct][128][KB*MC]
        at_b = np.ascontiguousarray(
            at_c.reshape(JT // KB, KB, 128, NCH, MC)
            .transpose(3, 0, 2, 1, 4)
            .reshape(NCH, JT // KB, 128, KB * MC))

        ft_c = np.empty((F, N), dtype=np.float16)
        ft_c[:, :SH] = featT[:, r0:r1]
        ft_c[:, SH:SH + r0] = featT[:, :r0]
        ft_c[:, SH + r0:] = featT[:, r1:]

        hd_c = np.concatenate([hat_d[r0:r1], hat_d[:r0], hat_d[r1:]])
        hdt_c = np.ascontiguousarray(hd_c.reshape(JT, 128).T)
        hdo_c = np.ascontiguousarray(hat_d[r0:r1].reshape(1, SH))

        in_maps.append({
            "at": at_b,
            "ft": ft_c,
            "hdt": hdt_c,
            "hdo": hdo_c,
            "wt": wt,
            "bvec": b2,
        })
    return in_maps


last_exec_time_ns = None
last_results = None


def kernel(A, hat_d, feature, W, b):
    global last_exec_time_ns, last_results
    if "nc" not in _CACHE:
        _CACHE["nc"] = build_program()
    nc = _CACHE["nc"]

    in_maps = prep_inputs(A, hat_d, feature, W, b)
    trace = bool(int(os.environ.get("KERNEL_TRACE", "0")))
    res = run_bass_kernel_spmd(nc, in_maps, list(range(NCORES)), trace=trace)
    last_exec_time_ns = res.exec_time_ns
    last_results = res

    out = np.empty((N, O), dtype=np.float32)
    for c in range(NCORES):
        out[c * SH:(c + 1) * SH] = res.results[c]["outT"].T.astype(np.float32)
    return out
